# revision 3
# baseline (speedup 1.0000x reference)
"""AnatomicalSSIMLoss on 8 TRN2 NeuronCores, data-parallel over the batch.

Per core (16 samples):
  mask  = erode3(dilate31(erode29... actually erode3 -> dilate31 -> erode29)) of (Y > 5e-5)
          W-direction passes: VectorE bf16 min/max doubling (exact on binary data)
          H-direction passes: TensorE banded-ones matmuls (exact integer counts in
          f32 PSUM) + compare-binarize against in-bounds neighbor counts.
  box7  = valid 7x7 box sums of {Xm, Ym, Xm^2+Ym^2, Xm*Ym}:
          W-pass 4-add doubling on VectorE (bf16), H-pass banded matmul on TensorE
          (bf16 in, exact f32 accumulate), scale folded into the PSUM->SBUF copy.
  SSIM  = fused tensor_scalar / scalar_tensor_tensor ops; 1/den via exp(-ln(den))
          on ScalarE; per-sample sums accumulated on-chip, summed on host.
"""
import numpy as np
import ml_dtypes

import concourse.bacc as bacc
import concourse.mybir as mybir
import concourse.tile as tile
from concourse.bass_utils import run_bass_kernel_spmd

F32 = mybir.dt.float32
BF16 = mybir.dt.bfloat16
AF = mybir.ActivationFunctionType
OP = mybir.AluOpType

B, H, W = 128, 384, 384
NCORES = 8
SPC = B // NCORES          # samples per core
WOUT = 378                 # 384 - 7 + 1
CN = 49.0 / 48.0           # unbiased covariance norm
THRESH = 5e-5
NPIX = B * WOUT * WOUT

# (M-chunk -> K-block list) for the banded matmuls
MORPH_KBS = {c: [k for k in (c - 1, c, c + 1) if 0 <= k < 3] for c in range(3)}
BOX_KBS = {0: [0, 1], 1: [0, 1, 2], 2: [1, 2]}


def build_nc():
    nc = bacc.Bacc(None, target_bir_lowering=False, debug=False)
    Xd = nc.declare_dram_parameter("X", [SPC, H, W], F32, isOutput=False)
    Yd = nc.declare_dram_parameter("Y", [SPC, H, W], F32, isOutput=False)
    DRd = nc.declare_dram_parameter("dr", [1, SPC], F32, isOutput=False)
    B3d = nc.declare_dram_parameter("band3", [H, W], BF16, isOutput=False)
    B31d = nc.declare_dram_parameter("band31", [H, W], BF16, isOutput=False)
    B29d = nc.declare_dram_parameter("band29", [H, W], BF16, isOutput=False)
    B7d = nc.declare_dram_parameter("band7", [H, WOUT], BF16, isOutput=False)
    C3d = nc.declare_dram_parameter("cnt3m", [128, 3], F32, isOutput=False)
    C29d = nc.declare_dram_parameter("cnt29m", [128, 3], F32, isOutput=False)
    OUTd = nc.declare_dram_parameter("out", [128, SPC], F32, isOutput=True)

    with tile.TileContext(nc) as tc:
        with (
            tc.tile_pool(name="const", bufs=1) as constp,
            tc.tile_pool(name="io", bufs=2) as iop,
            tc.tile_pool(name="mk", bufs=2) as mkp,
            tc.tile_pool(name="mscr", bufs=4) as mscrp,
            tc.tile_pool(name="maps", bufs=2) as mapp,
            tc.tile_pool(name="dd", bufs=2) as ddp,
            tc.tile_pool(name="mpsum", bufs=1, space="PSUM") as mps,
            tc.tile_pool(name="cpsum", bufs=4, space="PSUM") as cps,
            tc.tile_pool(name="dpsum", bufs=1, space="PSUM") as dps,
        ):
            # ---------------- constants ----------------
            b3 = constp.tile([128, 3, W], BF16, tag="b3")
            nc.sync.dma_start(b3[:], B3d.ap().rearrange("(b p) i -> p b i", p=128))
            b31 = constp.tile([128, 3, W], BF16, tag="b31")
            nc.sync.dma_start(b31[:], B31d.ap().rearrange("(b p) i -> p b i", p=128))
            b29 = constp.tile([128, 3, W], BF16, tag="b29")
            nc.sync.dma_start(b29[:], B29d.ap().rearrange("(b p) i -> p b i", p=128))
            b7 = constp.tile([128, 3, WOUT], BF16, tag="b7")
            nc.sync.dma_start(b7[:], B7d.ap().rearrange("(b p) i -> p b i", p=128))
            c3 = constp.tile([128, 3], F32, tag="c3")
            nc.sync.dma_start(c3[:], C3d.ap())
            c29 = constp.tile([128, 3], F32, tag="c29")
            nc.sync.dma_start(c29[:], C29d.ap())

            acc = constp.tile([128, SPC], F32, tag="acc")
            nc.vector.memset(acc[:], 0.0)

            # data_range -> per-partition-broadcast C1, C2 via rank-1 matmul
            drs = constp.tile([1, SPC], F32, tag="drs")
            nc.sync.dma_start(drs[:], DRd.ap())
            ones = constp.tile([1, 128], F32, tag="ones")
            nc.vector.memset(ones[:], 1.0)
            drp = dps.tile([128, SPC], F32, tag="drp")
            nc.tensor.matmul(drp[:], ones[:], drs[:], start=True, stop=True)
            drb = constp.tile([128, SPC], F32, tag="drb")
            nc.vector.tensor_copy(drb[:], drp[:])
            dr2 = constp.tile([128, SPC], F32, tag="dr2")
            nc.vector.tensor_mul(dr2[:], drb[:], drb[:])
            C1t = constp.tile([128, SPC], F32, tag="C1t")
            nc.vector.tensor_scalar_mul(C1t[:], dr2[:], 1e-4)
            C2t = constp.tile([128, SPC], F32, tag="C2t")
            nc.vector.tensor_scalar_mul(C2t[:], dr2[:], 9e-4)

            for s in range(SPC):
                xt = iop.tile([128, 3, W], F32, tag="xt")
                yt = iop.tile([128, 3, W], F32, tag="yt")
                nc.sync.dma_start(xt[:], Xd[s].rearrange("(t p) w -> p t w", p=128))
                nc.sync.dma_start(yt[:], Yd[s].rearrange("(t p) w -> p t w", p=128))

                # ---------------- mask ----------------
                # threshold + erode3 W (1-padded)
                mp_ = mkp.tile([128, 3, 386], BF16, tag="mpad")
                nc.gpsimd.memset(mp_[:, :, 0:1], 1.0)
                nc.gpsimd.memset(mp_[:, :, 385:386], 1.0)
                nc.gpsimd.tensor_scalar(mp_[:, :, 1:385], yt[:], THRESH, None, op0=OP.is_gt)
                t1 = mscrp.tile([128, 3, 414], BF16, tag="ms")
                nc.vector.tensor_tensor(t1[:, :, 0:385], mp_[:, :, 0:385], mp_[:, :, 1:386], op=OP.min)
                er3w = mscrp.tile([128, 3, 414], BF16, tag="ms")
                nc.vector.tensor_tensor(er3w[:, :, 0:384], t1[:, :, 0:384], t1[:, :, 1:385], op=OP.min)

                # erode3 H on TensorE + binarize
                ps1 = mps.tile([128, 3, 512], F32, tag="mp")
                for c in range(3):
                    kbs = MORPH_KBS[c]
                    for i, kb in enumerate(kbs):
                        nc.tensor.matmul(ps1[:, c, 0:W], b3[:, kb, 128 * c:128 * c + 128],
                                         er3w[:, kb, 0:W],
                                         start=(i == 0), stop=(i == len(kbs) - 1))
                m1 = mkp.tile([128, 3, W], BF16, tag="m1")
                for c in range(3):
                    nc.vector.tensor_scalar(m1[:, c, :], ps1[:, c, 0:W], c3[:, c:c + 1], None, op0=OP.is_ge)

                # dilate31 H on TensorE + binarize into 0-padded buffer
                ps2 = mps.tile([128, 3, 512], F32, tag="mp")
                for c in range(3):
                    kbs = MORPH_KBS[c]
                    for i, kb in enumerate(kbs):
                        nc.tensor.matmul(ps2[:, c, 0:W], b31[:, kb, 128 * c:128 * c + 128],
                                         m1[:, kb, :],
                                         start=(i == 0), stop=(i == len(kbs) - 1))
                d1 = mkp.tile([128, 3, 414], BF16, tag="d1")
                nc.gpsimd.memset(d1[:, :, 0:15], 0.0)
                nc.gpsimd.memset(d1[:, :, 399:414], 0.0)
                for c in range(3):
                    nc.vector.tensor_scalar(d1[:, c, 15:399], ps2[:, c, 0:W], 0.5, None, op0=OP.is_ge)

                # dilate31 W doubling (max over 31)
                M2 = mscrp.tile([128, 3, 414], BF16, tag="ms")
                nc.vector.tensor_tensor(M2[:, :, 0:413], d1[:, :, 0:413], d1[:, :, 1:414], op=OP.max)
                M4 = mscrp.tile([128, 3, 414], BF16, tag="ms")
                nc.vector.tensor_tensor(M4[:, :, 0:411], M2[:, :, 0:411], M2[:, :, 2:413], op=OP.max)
                M8 = mscrp.tile([128, 3, 414], BF16, tag="ms")
                nc.vector.tensor_tensor(M8[:, :, 0:407], M4[:, :, 0:407], M4[:, :, 4:411], op=OP.max)
                M16 = mscrp.tile([128, 3, 414], BF16, tag="ms")
                nc.vector.tensor_tensor(M16[:, :, 0:399], M8[:, :, 0:399], M8[:, :, 8:407], op=OP.max)
                e0 = mkp.tile([128, 3, 412], BF16, tag="e0")
                nc.gpsimd.memset(e0[:, :, 0:14], 1.0)
                nc.gpsimd.memset(e0[:, :, 398:412], 1.0)
                nc.vector.tensor_tensor(e0[:, :, 14:398], M16[:, :, 0:384], M16[:, :, 15:399], op=OP.max)

                # erode29 W doubling (min over 29)
                N2 = mscrp.tile([128, 3, 414], BF16, tag="ms")
                nc.vector.tensor_tensor(N2[:, :, 0:411], e0[:, :, 0:411], e0[:, :, 1:412], op=OP.min)
                N4 = mscrp.tile([128, 3, 414], BF16, tag="ms")
                nc.vector.tensor_tensor(N4[:, :, 0:409], N2[:, :, 0:409], N2[:, :, 2:411], op=OP.min)
                N8 = mscrp.tile([128, 3, 414], BF16, tag="ms")
                nc.vector.tensor_tensor(N8[:, :, 0:405], N4[:, :, 0:405], N4[:, :, 4:409], op=OP.min)
                N16 = mscrp.tile([128, 3, 414], BF16, tag="ms")
                nc.vector.tensor_tensor(N16[:, :, 0:397], N8[:, :, 0:397], N8[:, :, 8:405], op=OP.min)
                e29 = mkp.tile([128, 3, W], BF16, tag="e29")
                nc.vector.tensor_tensor(e29[:, :, 0:384], N16[:, :, 0:384], N16[:, :, 13:397], op=OP.min)

                # erode29 H on TensorE + binarize -> mask
                ps3 = mps.tile([128, 3, 512], F32, tag="mp")
                for c in range(3):
                    kbs = MORPH_KBS[c]
                    for i, kb in enumerate(kbs):
                        nc.tensor.matmul(ps3[:, c, 0:W], b29[:, kb, 128 * c:128 * c + 128],
                                         e29[:, kb, 0:W],
                                         start=(i == 0), stop=(i == len(kbs) - 1))
                msk = mkp.tile([128, 3, W], BF16, tag="msk")
                for c in range(3):
                    nc.vector.tensor_scalar(msk[:, c, :], ps3[:, c, 0:W], c29[:, c:c + 1], None, op0=OP.is_ge)

                # ---------------- products ----------------
                xc = mapp.tile([128, 3, W], BF16, tag="xc")
                nc.gpsimd.tensor_copy(xc[:], xt[:])
                yc = mapp.tile([128, 3, W], BF16, tag="yc")
                nc.gpsimd.tensor_copy(yc[:], yt[:])
                xm = mapp.tile([128, 3, W], BF16, tag="xm")
                nc.vector.tensor_mul(xm[:], xc[:], msk[:])
                ym = mapp.tile([128, 3, W], BF16, tag="ym")
                nc.vector.tensor_mul(ym[:], yc[:], msk[:])
                sqx = mapp.tile([128, 3, W], BF16, tag="sqx")
                nc.gpsimd.tensor_mul(sqx[:], xm[:], xm[:])
                sqy = mapp.tile([128, 3, W], BF16, tag="sqy")
                nc.gpsimd.tensor_mul(sqy[:], ym[:], ym[:])
                zz = mapp.tile([128, 3, W], BF16, tag="zz")
                nc.vector.tensor_add(zz[:], sqx[:], sqy[:])
                xy = mapp.tile([128, 3, W], BF16, tag="xy")
                nc.vector.tensor_mul(xy[:], xm[:], ym[:])

                # ---------------- box filters + PSUM exit ----------------
                us = {}
                specs = [("ux", xm, 1.0 / 49, 0.0), ("uy", ym, 1.0 / 49, 0.0),
                         ("tz", zz, CN / 49, C2t), ("tw", xy, 2 * CN / 49, C2t)]
                for name, src, scale, bias in specs:
                    s2 = mscrp.tile([128, 3, 414], BF16, tag="ms")
                    nc.vector.tensor_add(s2[:, :, 0:383], src[:, :, 0:383], src[:, :, 1:384])
                    s4 = mscrp.tile([128, 3, 414], BF16, tag="ms")
                    nc.vector.tensor_add(s4[:, :, 0:381], s2[:, :, 0:381], s2[:, :, 2:383])
                    s6 = mscrp.tile([128, 3, 414], BF16, tag="ms")
                    nc.vector.tensor_add(s6[:, :, 0:379], s4[:, :, 0:379], s2[:, :, 4:383])
                    s7 = mapp.tile([128, 3, W], BF16, tag="s7")
                    nc.vector.tensor_add(s7[:, :, 0:WOUT], s6[:, :, 0:WOUT], src[:, :, 6:384])

                    ut = ddp.tile([126, 3, W], BF16, tag="u" + name)
                    for c in range(3):
                        cp = cps.tile([126, 512], F32, tag="cp")
                        kbs = BOX_KBS[c]
                        for i, kb in enumerate(kbs):
                            nc.tensor.matmul(cp[:, 0:WOUT], b7[:, kb, 126 * c:126 * c + 126],
                                             s7[:, kb, 0:WOUT],
                                             start=(i == 0), stop=(i == len(kbs) - 1))
                        bias_ap = bias if isinstance(bias, float) else bias[0:126, s:s + 1]
                        nc.scalar.activation(ut[:, c, 0:WOUT], cp[:, 0:WOUT], AF.Identity,
                                             bias=bias_ap, scale=scale)
                    us[name] = ut

                # ---------------- SSIM pointwise ----------------
                def sl(t):
                    return t[0:126, :, 0:WOUT]

                ux, uy, tz, tw = us["ux"], us["uy"], us["tz"], us["tw"]
                pp = ddp.tile([126, 3, W], BF16, tag="pp")
                nc.vector.tensor_mul(sl(pp), sl(ux), sl(uy))
                a2 = ddp.tile([126, 3, W], BF16, tag="a2")
                nc.vector.scalar_tensor_tensor(sl(a2), sl(pp), -2 * CN, sl(tw), op0=OP.mult, op1=OP.add)
                sx = ddp.tile([126, 3, W], BF16, tag="sx")
                nc.vector.tensor_mul(sl(sx), sl(ux), sl(ux))
                sy = ddp.tile([126, 3, W], BF16, tag="sy")
                nc.vector.tensor_mul(sl(sy), sl(uy), sl(uy))
                qq = ddp.tile([126, 3, W], BF16, tag="qq")
                nc.vector.tensor_add(sl(qq), sl(sx), sl(sy))
                b2t = ddp.tile([126, 3, W], BF16, tag="b2t")
                nc.vector.scalar_tensor_tensor(sl(b2t), sl(qq), -CN, sl(tz), op0=OP.mult, op1=OP.add)
                a1 = ddp.tile([126, 3, W], BF16, tag="a1")
                nc.vector.tensor_scalar(sl(a1), sl(pp), 2.0, C1t[0:126, s:s + 1], op0=OP.mult, op1=OP.add)
                b1 = ddp.tile([126, 3, W], BF16, tag="b1")
                nc.vector.tensor_scalar(sl(b1), sl(qq), C1t[0:126, s:s + 1], None, op0=OP.add)
                num = ddp.tile([126, 3, W], BF16, tag="num")
                nc.vector.tensor_mul(sl(num), sl(a1), sl(a2))
                den = ddp.tile([126, 3, W], BF16, tag="den")
                nc.vector.tensor_mul(sl(den), sl(b1), sl(b2t))
                ld = ddp.tile([126, 3, W], BF16, tag="ld")
                nc.scalar.activation(sl(ld), sl(den), AF.Ln)
                rd = ddp.tile([126, 3, W], BF16, tag="rd")
                nc.scalar.activation(sl(rd), sl(ld), AF.Exp, scale=-1.0)
                ss = ddp.tile([126, 3, W], BF16, tag="ss")
                nc.vector.scalar_tensor_tensor(sl(ss), sl(num), 1.0, sl(rd),
                                               op0=OP.bypass, op1=OP.mult,
                                               accum_out=acc[0:126, s:s + 1])

            nc.sync.dma_start(OUTd.ap(), acc[:])

    nc.finalize()
    return nc


def _host_constants():
    idx = np.arange(H)
    k, i = np.meshgrid(idx, idx, indexing="ij")
    band3 = (np.abs(k - i) <= 1).astype(ml_dtypes.bfloat16)
    band31 = (np.abs(k - i) <= 15).astype(ml_dtypes.bfloat16)
    band29 = (np.abs(k - i) <= 14).astype(ml_dtypes.bfloat16)
    o = np.arange(WOUT)
    k2, o2 = np.meshgrid(idx, o, indexing="ij")
    band7 = ((k2 >= o2) & (k2 <= o2 + 6)).astype(ml_dtypes.bfloat16)

    r = idx.astype(np.float64)
    cnt3 = np.minimum(H - 1, r + 1) - np.maximum(0, r - 1) + 1
    cnt29 = np.minimum(H - 1, r + 14) - np.maximum(0, r - 14) + 1
    cnt3m = (cnt3 - 0.5).reshape(3, 128).T.astype(np.float32).copy()
    cnt29m = (cnt29 - 0.5).reshape(3, 128).T.astype(np.float32).copy()
    return band3, band31, band29, band7, cnt3m, cnt29m


_NC = None


def kernel(X: np.ndarray, Y: np.ndarray, data_range: np.ndarray) -> np.ndarray:
    global _NC
    if _NC is None:
        _NC = build_nc()
    band3, band31, band29, band7, cnt3m, cnt29m = _host_constants()
    in_maps = []
    for c in range(NCORES):
        s0, s1 = c * SPC, (c + 1) * SPC
        in_maps.append({
            "X": np.ascontiguousarray(X[s0:s1]).astype(np.float32),
            "Y": np.ascontiguousarray(Y[s0:s1]).astype(np.float32),
            "dr": np.ascontiguousarray(data_range[s0:s1]).astype(np.float32).reshape(1, SPC),
            "band3": band3, "band31": band31, "band29": band29, "band7": band7,
            "cnt3m": cnt3m, "cnt29m": cnt29m,
        })
    res = run_bass_kernel_spmd(_NC, in_maps, list(range(NCORES)))
    total = np.float64(0.0)
    for c in range(NCORES):
        total += np.float64(res.results[c]["out"].astype(np.float64).sum())
    return np.float32(1.0 - total / NPIX)
